# revision 6
# baseline (speedup 1.0000x reference)
"""Hard-negative contrastive loss on 8 TRN2 NeuronCores (Bass/Tile).

Reference semantics (B=1024, Q=32, D=512, temp scalar):
    sim[i,j,q] = fusion[i] . target[j,q];  v[i,j] = max_q sim / temp
    loss = mean_i(lse_j(v[i,:]) - v[i,i])
         + 0.5 * mean_i(log(exp(pos) + sum exp(top512 offdiag)) - pos)

Sharding: target rows j are split 128/core. Each core computes its
(1024 x 128) column block of v with fp8e4m3 DoubleRow matmuls
(sqrt(1/temp) folded into both operands host-side; d on partitions,
two 128-chunk pairs contracted per instruction; operands arrive in
partition-major layout so DMAs are contiguous). The Q-max runs on DVE
into fp32. The block exchange is TWO AllToAlls on bf16 j-halves: the
jq loop runs outermost, so the first half fires at the phase-1
midpoint and overlaps remaining matmuls (absorbing inter-core launch
skew); the second fires at the end. Per-row logsumexp and an
approximate top-512 threshold (6-step bisection; borderline mass
folded in at exp(hi)) reduce each row to 6 stats; the host finishes
the per-row losses and averages. Rel err ~1e-4 (gate is 2e-2).
"""
import sys

if "/opt/trn_rl_repo" not in sys.path:
    sys.path.insert(0, "/opt/trn_rl_repo")

import numpy as np

N_CORES = 8
B, Q, D = 1024, 32, 512
JQ = (B // N_CORES) * Q        # 4096 target vectors per core
NBLK = 512                     # jq per matmul / psum tile
JBLK = NBLK // Q               # 16 j columns per psum tile
NB = JQ // NBLK                # 8 jq blocks
N_ITERS = 6                    # bisection steps
NUM_HARD = B // 2              # 512
NEG_BIG = -1.0e30

_RUNNER = None


def _build():
    import concourse.bacc as bacc
    import concourse.mybir as mybir
    import concourse.tile as tile

    f32 = mybir.dt.float32
    f8 = mybir.dt.float8e4
    bf16 = mybir.dt.bfloat16
    i32 = mybir.dt.int32
    Alu = mybir.AluOpType
    Act = mybir.ActivationFunctionType
    X = mybir.AxisListType.X
    DR = mybir.MatmulPerfMode.DoubleRow

    nc = bacc.Bacc(None, target_bir_lowering=False, debug=False,
                   num_devices=N_CORES)

    fus_ap = nc.dram_tensor("fus8", [128, 2, 2, B], f8, kind="ExternalInput").ap()
    tgt_ap = nc.dram_tensor("tgt8", [128, 2, 2, JQ], f8, kind="ExternalInput").ap()
    oneh_ap = nc.dram_tensor("onehot", [128, B], bf16, kind="ExternalInput").ap()
    out_ap = nc.dram_tensor("rowstats", [128, 6], f32, kind="ExternalOutput").ap()

    with tile.TileContext(nc) as tc:
        with (
            tc.tile_pool(name="fus", bufs=1) as fus_pool,
            tc.tile_pool(name="tgt", bufs=1) as tgt_pool,
            tc.tile_pool(name="res", bufs=1) as res_pool,
            tc.tile_pool(name="big", bufs=1) as big_pool,
            tc.tile_pool(name="small", bufs=1) as small_pool,
            tc.tile_pool(name="psum", bufs=8, space="PSUM") as psum_pool,
            tc.tile_pool(name="dram", bufs=1, space="DRAM") as dram_pool,
        ):
            # ---------- phase 1: my (1024 x 128) block of v ----------
            fus = fus_pool.tile([128, 2, 2, B], f8)
            nc.sync.dma_start(fus[:], fus_ap[:])
            tgt = tgt_pool.tile([128, 2, 2, JQ], f8)
            for kp in range(2):
                for q4 in range(4):
                    nc.sync.dma_start(
                        tgt[:, kp, :, q4 * (JQ // 4):(q4 + 1) * (JQ // 4)],
                        tgt_ap[:, kp, :, q4 * (JQ // 4):(q4 + 1) * (JQ // 4)])
            oneh = big_pool.tile([128, B], bf16)
            nc.sync.dma_start(oneh[:], oneh_ap[:])

            P32 = res_pool.tile([128, N_CORES, 128], f32)   # [i_part, i_tile, j]
            Pb = res_pool.tile([128, N_CORES, 128], bf16)
            p_in = [dram_pool.tile([B, 64], bf16, name=f"p_in{h}", tag=f"p_in{h}")
                    for h in range(2)]
            p_out = [dram_pool.tile([B, 64], bf16, name=f"p_out{h}", tag=f"p_out{h}")
                     for h in range(2)]

            V = big_pool.tile([128, B], bf16)

            def exchange_half(h):
                # stage j-columns [h*64, h*64+64) of every i-tile, AllToAll
                lo_j, hi_j = h * 64, (h + 1) * 64
                nc.scalar.copy(Pb[:, :, lo_j:hi_j], P32[:, :, lo_j:hi_j])
                for it in range(N_CORES):
                    nc.sync.dma_start(p_in[h][it * 128:(it + 1) * 128, :],
                                      Pb[:, it, lo_j:hi_j])
                nc.gpsimd.collective_compute(
                    "AllToAll",
                    Alu.bypass,
                    replica_groups=[list(range(N_CORES))],
                    ins=[p_in[h].opt()],
                    outs=[p_out[h].opt()],
                )
                for s in range(N_CORES):
                    nc.sync.dma_start(
                        V[:, s * 128 + lo_j:s * 128 + hi_j],
                        p_out[h][s * 128:(s + 1) * 128, :])

            for b in range(NB):
                for it in range(N_CORES):
                    ps = psum_pool.tile([128, NBLK], f32)
                    for kp in range(2):
                        nc.tensor.matmul(
                            ps[:],
                            fus[:, kp, :, it * 128:(it + 1) * 128],
                            tgt[:, kp, :, b * NBLK:(b + 1) * NBLK],
                            start=(kp == 0),
                            stop=(kp == 1),
                            perf_mode=DR,
                        )
                    nc.vector.reduce_max(
                        P32[:, it, b * JBLK:(b + 1) * JBLK],
                        ps.rearrange("p (j q) -> p j q", q=Q),
                        axis=X,
                    )
                if b == NB // 2 - 1:
                    exchange_half(0)
            exchange_half(1)

            # ---------- phase 2: per-row stats ----------
            Vmask = big_pool.tile([128, B], f32)
            E = big_pool.tile([128, B], bf16)
            junk = big_pool.tile([128, B], bf16)
            junkf = big_pool.tile([128, B], f32)

            outs = res_pool.tile([128, 6], f32)  # m pos sumfull sumsel cnt_hi hi
            m = outs[:, 0:1]
            pos = outs[:, 1:2]
            sumfull = outs[:, 2:3]
            sumsel = outs[:, 3:4]
            cnt_hi = outs[:, 4:5]

            def sm(name, dt=f32):
                return small_pool.tile([128, 1], dt, name=name, tag=name)

            negm, lo, hi, mid, cnt = (sm(n) for n in "negm lo hi mid cnt".split())
            upd = sm("upd", i32)
            updn = sm("updn", i32)

            nc.vector.reduce_max(m, V[:], axis=X)
            nc.vector.tensor_scalar_mul(negm[:], m, -1.0)
            nc.vector.tensor_reduce(lo[:], V[:], axis=X, op=Alu.min)
            nc.vector.tensor_scalar_add(lo[:], lo[:], -1.0)
            nc.vector.tensor_copy(hi[:], m)

            # Vmask = V - 1e30 * onehot
            nc.vector.scalar_tensor_tensor(
                Vmask[:], oneh[:], NEG_BIG, V[:], op0=Alu.mult, op1=Alu.add)

            # E = exp(V - m), sumfull = sum_j E  (scalar engine, concurrent)
            nc.scalar.activation(E[:], V[:], Act.Exp, bias=negm[:], scale=1.0,
                                 accum_out=sumfull)

            # bisection for the top-512 threshold
            for _ in range(N_ITERS):
                nc.vector.tensor_add(mid[:], lo[:], hi[:])
                nc.vector.tensor_scalar_mul(mid[:], mid[:], 0.5)
                nc.vector.tensor_scalar(
                    junkf[:], Vmask[:], mid[:], None, op0=Alu.is_gt,
                    op1=Alu.add, accum_out=cnt[:])
                nc.vector.tensor_scalar(upd[:], cnt[:], float(NUM_HARD), None,
                                        op0=Alu.is_gt)
                nc.vector.tensor_scalar(updn[:], cnt[:], float(NUM_HARD), None,
                                        op0=Alu.is_le)
                nc.vector.copy_predicated(lo[:], upd[:], mid[:])
                nc.vector.copy_predicated(hi[:], updn[:], mid[:])

            # pos = sum(onehot * V)  (off the bisection chain)
            nc.vector.scalar_tensor_tensor(
                junk[:], oneh[:], 1.0, V[:], op0=Alu.mult, op1=Alu.mult,
                accum_out=pos)
            # cnt_hi = #{v > hi};  sumsel = sum E over those entries
            nc.vector.tensor_scalar(
                junkf[:], Vmask[:], hi[:], None, op0=Alu.is_gt,
                op1=Alu.add, accum_out=cnt_hi)
            nc.vector.scalar_tensor_tensor(
                junkf[:], Vmask[:], hi[:], E[:], op0=Alu.is_gt, op1=Alu.mult,
                accum_out=sumsel)
            nc.vector.tensor_copy(outs[:, 5:6], hi[:])

            nc.sync.dma_start(out_ap[:], outs[:])

    nc.compile()
    return nc


def _get_nc():
    global _RUNNER
    if _RUNNER is None:
        _RUNNER = _build()
    return _RUNNER


def make_in_maps(fusion_feats, target_feats, temp):
    import ml_dtypes

    f8 = ml_dtypes.float8_e4m3
    fusion = np.asarray(fusion_feats, dtype=np.float32)
    target = np.asarray(target_feats, dtype=np.float32)
    scale = np.float32(1.0 / np.sqrt(float(np.asarray(temp))))
    # d -> (kp, pair, p): d = kp*256 + pair*128 + p; partition-major layout
    fus8 = np.ascontiguousarray(
        (fusion * scale).T.reshape(2, 2, 128, B).transpose(2, 0, 1, 3)
    ).astype(f8)
    rows_per = B // N_CORES
    in_maps = []
    for c in range(N_CORES):
        shard = target[c * rows_per:(c + 1) * rows_per].reshape(JQ, D)
        tgt8 = np.ascontiguousarray(
            (shard * scale).T.reshape(2, 2, 128, JQ).transpose(2, 0, 1, 3)
        ).astype(f8)
        onehot = np.zeros((rows_per, B), dtype=ml_dtypes.bfloat16)
        onehot[np.arange(rows_per), c * rows_per + np.arange(rows_per)] = 1.0
        in_maps.append({"fus8": fus8, "tgt8": tgt8, "onehot": onehot})
    return in_maps


def combine(results):
    rows = np.concatenate([r["rowstats"] for r in results], axis=0)  # (1024, 6)
    m, pos, sumfull, sumsel, cnt_hi, hi = (rows[:, k].astype(np.float64)
                                           for k in range(6))
    epos = np.exp(pos - m)
    ehi = np.exp(hi - m)
    acc = epos + sumsel + (NUM_HARD - cnt_hi) * ehi
    loss_std = (m + np.log(sumfull) - pos).mean()
    loss_hard = (m + np.log(acc) - pos).mean()
    return np.asarray(loss_std + 0.5 * loss_hard, dtype=np.float32)


def kernel(fusion_feats, target_feats, temp):
    from concourse import bass_utils

    nc = _get_nc()
    in_maps = make_in_maps(fusion_feats, target_feats, temp)
    res = bass_utils.run_bass_kernel_spmd(nc, in_maps, list(range(N_CORES)))
    return combine(res.results)


# revision 7
# speedup vs baseline: 1.1189x; 1.1189x over previous
"""Hard-negative contrastive loss on 8 TRN2 NeuronCores (Bass/Tile).

Reference semantics (B=1024, Q=32, D=512, temp scalar):
    sim[i,j,q] = fusion[i] . target[j,q];  v[i,j] = max_q sim / temp
    loss = mean_i(lse_j(v[i,:]) - v[i,i])
         + 0.5 * mean_i(log(exp(pos) + sum exp(top512 offdiag)) - pos)

Sharding: target rows j are split 128/core. Each core computes its
(1024 x 128) column block of v with fp8e4m3 DoubleRow matmuls
(sqrt(1/temp) folded into both operands host-side; d on partitions,
two 128-chunk pairs contracted per instruction; operands arrive in
partition-major layout so DMAs are contiguous). The Q-max runs on DVE
into fp32. The block exchange is TWO AllToAlls on bf16 j-halves: the
jq loop runs outermost, so the first half fires at the phase-1
midpoint and overlaps remaining matmuls (absorbing inter-core launch
skew); the second fires at the end. Per-row logsumexp and an
approximate top-512 threshold (6-step bisection; borderline mass
folded in at exp(hi)) reduce each row to 6 stats; the host finishes
the per-row losses and averages. Rel err ~1e-4 (gate is 2e-2).
"""
import sys

if "/opt/trn_rl_repo" not in sys.path:
    sys.path.insert(0, "/opt/trn_rl_repo")

import numpy as np

N_CORES = 8
B, Q, D = 1024, 32, 512
JQ = (B // N_CORES) * Q        # 4096 target vectors per core
NBLK = 512                     # jq per matmul / psum tile
JBLK = NBLK // Q               # 16 j columns per psum tile
NB = JQ // NBLK                # 8 jq blocks
N_ITERS = 6                    # bisection steps
NUM_HARD = B // 2              # 512
NEG_BIG = -1.0e30

_RUNNER = None


def _build():
    import concourse.bacc as bacc
    import concourse.mybir as mybir
    import concourse.tile as tile

    f32 = mybir.dt.float32
    f8 = mybir.dt.float8e4
    bf16 = mybir.dt.bfloat16
    i32 = mybir.dt.int32
    Alu = mybir.AluOpType
    Act = mybir.ActivationFunctionType
    X = mybir.AxisListType.X
    DR = mybir.MatmulPerfMode.DoubleRow

    nc = bacc.Bacc(None, target_bir_lowering=False, debug=False,
                   num_devices=N_CORES)

    fus_ap = nc.dram_tensor("fus8", [128, 2, 2, B], f8, kind="ExternalInput").ap()
    tgt_ap = nc.dram_tensor("tgt8", [128, 2, 2, JQ], f8, kind="ExternalInput").ap()
    oneh_ap = nc.dram_tensor("onehot", [128, B], bf16, kind="ExternalInput").ap()
    out_ap = nc.dram_tensor("rowstats", [128, 6], f32, kind="ExternalOutput").ap()

    with tile.TileContext(nc) as tc:
        with (
            tc.tile_pool(name="fus", bufs=1) as fus_pool,
            tc.tile_pool(name="tgt", bufs=1) as tgt_pool,
            tc.tile_pool(name="res", bufs=1) as res_pool,
            tc.tile_pool(name="big", bufs=1) as big_pool,
            tc.tile_pool(name="small", bufs=1) as small_pool,
            tc.tile_pool(name="psum", bufs=8, space="PSUM") as psum_pool,
            tc.tile_pool(name="dram", bufs=1, space="DRAM") as dram_pool,
        ):
            # ---------- phase 1: my (1024 x 128) block of v ----------
            fus = fus_pool.tile([128, 2, 2, B], f8)
            nc.sync.dma_start(fus[:], fus_ap[:])
            tgt = tgt_pool.tile([128, 2, 2, JQ], f8)
            for kp in range(2):
                for q4 in range(4):
                    nc.sync.dma_start(
                        tgt[:, kp, :, q4 * (JQ // 4):(q4 + 1) * (JQ // 4)],
                        tgt_ap[:, kp, :, q4 * (JQ // 4):(q4 + 1) * (JQ // 4)])
            oneh = big_pool.tile([128, B], bf16)
            nc.sync.dma_start(oneh[:], oneh_ap[:])

            P32 = res_pool.tile([128, N_CORES, 128], f32)   # [i_part, i_tile, j]
            Pb = res_pool.tile([128, N_CORES, 128], bf16)
            p_in = dram_pool.tile([B, 128], bf16)
            p_out = dram_pool.tile([B, 128], bf16)

            for it in range(N_CORES):
                for b in range(NB):
                    ps = psum_pool.tile([128, NBLK], f32)
                    for kp in range(2):
                        nc.tensor.matmul(
                            ps[:],
                            fus[:, kp, :, it * 128:(it + 1) * 128],
                            tgt[:, kp, :, b * NBLK:(b + 1) * NBLK],
                            start=(kp == 0),
                            stop=(kp == 1),
                            perf_mode=DR,
                        )
                    nc.vector.reduce_max(
                        P32[:, it, b * JBLK:(b + 1) * JBLK],
                        ps.rearrange("p (j q) -> p j q", q=Q),
                        axis=X,
                    )
                # cast this i-tile to bf16 (scalar engine) and stage it for
                # the AllToAll so exchange DMAs overlap the remaining tiles
                nc.scalar.copy(Pb[:, it], P32[:, it])
                nc.sync.dma_start(p_in[it * 128:(it + 1) * 128, :], Pb[:, it])

            nc.gpsimd.collective_compute(
                "AllToAll",
                Alu.bypass,
                replica_groups=[list(range(N_CORES))],
                ins=[p_in.opt()],
                outs=[p_out.opt()],
            )
            V = big_pool.tile([128, B], bf16)
            for s in range(N_CORES):
                nc.sync.dma_start(V[:, s * 128:(s + 1) * 128],
                                  p_out[s * 128:(s + 1) * 128, :])

            # ---------- phase 2: per-row stats ----------
            Vmask = big_pool.tile([128, B], f32)
            E = big_pool.tile([128, B], bf16)
            junk = big_pool.tile([128, B], bf16)
            junkf = big_pool.tile([128, B], f32)

            outs = res_pool.tile([128, 6], f32)  # m pos sumfull sumsel cnt_hi hi
            m = outs[:, 0:1]
            pos = outs[:, 1:2]
            sumfull = outs[:, 2:3]
            sumsel = outs[:, 3:4]
            cnt_hi = outs[:, 4:5]

            def sm(name, dt=f32):
                return small_pool.tile([128, 1], dt, name=name, tag=name)

            negm, lo, hi, mid, cnt = (sm(n) for n in "negm lo hi mid cnt".split())
            upd = sm("upd", i32)
            updn = sm("updn", i32)

            nc.vector.reduce_max(m, V[:], axis=X)
            nc.vector.tensor_scalar_mul(negm[:], m, -1.0)
            nc.vector.tensor_reduce(lo[:], V[:], axis=X, op=Alu.min)
            nc.vector.tensor_scalar_add(lo[:], lo[:], -1.0)
            nc.vector.tensor_copy(hi[:], m)

            # Vmask = V - 1e30 * onehot
            nc.vector.scalar_tensor_tensor(
                Vmask[:], oneh[:], NEG_BIG, V[:], op0=Alu.mult, op1=Alu.add)

            # E = exp(V - m), sumfull = sum_j E  (scalar engine, concurrent)
            nc.scalar.activation(E[:], V[:], Act.Exp, bias=negm[:], scale=1.0,
                                 accum_out=sumfull)

            # bisection for the top-512 threshold
            for _ in range(N_ITERS):
                nc.vector.tensor_add(mid[:], lo[:], hi[:])
                nc.vector.tensor_scalar_mul(mid[:], mid[:], 0.5)
                nc.vector.tensor_scalar(
                    junkf[:], Vmask[:], mid[:], None, op0=Alu.is_gt,
                    op1=Alu.add, accum_out=cnt[:])
                nc.vector.tensor_scalar(upd[:], cnt[:], float(NUM_HARD), None,
                                        op0=Alu.is_gt)
                nc.vector.tensor_scalar(updn[:], cnt[:], float(NUM_HARD), None,
                                        op0=Alu.is_le)
                nc.vector.copy_predicated(lo[:], upd[:], mid[:])
                nc.vector.copy_predicated(hi[:], updn[:], mid[:])

            # pos = sum(onehot * V)  (off the bisection chain)
            nc.vector.scalar_tensor_tensor(
                junk[:], oneh[:], 1.0, V[:], op0=Alu.mult, op1=Alu.mult,
                accum_out=pos)
            # cnt_hi = #{v > hi};  sumsel = sum E over those entries
            nc.vector.tensor_scalar(
                junkf[:], Vmask[:], hi[:], None, op0=Alu.is_gt,
                op1=Alu.add, accum_out=cnt_hi)
            nc.vector.scalar_tensor_tensor(
                junkf[:], Vmask[:], hi[:], E[:], op0=Alu.is_gt, op1=Alu.mult,
                accum_out=sumsel)
            nc.vector.tensor_copy(outs[:, 5:6], hi[:])

            nc.sync.dma_start(out_ap[:], outs[:])

    nc.compile()
    return nc


def _get_nc():
    global _RUNNER
    if _RUNNER is None:
        _RUNNER = _build()
    return _RUNNER


def make_in_maps(fusion_feats, target_feats, temp):
    import ml_dtypes

    f8 = ml_dtypes.float8_e4m3
    fusion = np.asarray(fusion_feats, dtype=np.float32)
    target = np.asarray(target_feats, dtype=np.float32)
    scale = np.float32(1.0 / np.sqrt(float(np.asarray(temp))))
    # d -> (kp, pair, p): d = kp*256 + pair*128 + p; partition-major layout
    fus8 = np.ascontiguousarray(
        (fusion * scale).T.reshape(2, 2, 128, B).transpose(2, 0, 1, 3)
    ).astype(f8)
    rows_per = B // N_CORES
    in_maps = []
    for c in range(N_CORES):
        shard = target[c * rows_per:(c + 1) * rows_per].reshape(JQ, D)
        tgt8 = np.ascontiguousarray(
            (shard * scale).T.reshape(2, 2, 128, JQ).transpose(2, 0, 1, 3)
        ).astype(f8)
        onehot = np.zeros((rows_per, B), dtype=ml_dtypes.bfloat16)
        onehot[np.arange(rows_per), c * rows_per + np.arange(rows_per)] = 1.0
        in_maps.append({"fus8": fus8, "tgt8": tgt8, "onehot": onehot})
    return in_maps


def combine(results):
    rows = np.concatenate([r["rowstats"] for r in results], axis=0)  # (1024, 6)
    m, pos, sumfull, sumsel, cnt_hi, hi = (rows[:, k].astype(np.float64)
                                           for k in range(6))
    epos = np.exp(pos - m)
    ehi = np.exp(hi - m)
    acc = epos + sumsel + (NUM_HARD - cnt_hi) * ehi
    loss_std = (m + np.log(sumfull) - pos).mean()
    loss_hard = (m + np.log(acc) - pos).mean()
    return np.asarray(loss_std + 0.5 * loss_hard, dtype=np.float32)


def kernel(fusion_feats, target_feats, temp):
    from concourse import bass_utils

    nc = _get_nc()
    in_maps = make_in_maps(fusion_feats, target_feats, temp)
    res = bass_utils.run_bass_kernel_spmd(nc, in_maps, list(range(N_CORES)))
    return combine(res.results)


# revision 10
# speedup vs baseline: 1.4555x; 1.3009x over previous
"""Hard-negative contrastive loss on 8 TRN2 NeuronCores (Bass/Tile).

Reference semantics (B=1024, Q=32, D=512, temp scalar):
    sim[i,j,q] = fusion[i] . target[j,q];  v[i,j] = max_q sim / temp
    loss = mean_i(lse_j(v[i,:]) - v[i,i])
         + 0.5 * mean_i(log(exp(pos) + sum exp(top512 offdiag)) - pos)

Sharding: OUTPUT rows i are split 128/core; the full fp8 target tensor
(16.8MB) is replicated to every core, so there is no inter-core
exchange at all (no collective, no skew coupling). Each core computes
its (128 x 1024) slab of v with fp8e4m3 DoubleRow matmuls
(sqrt(1/temp) folded into both operands host-side; d on partitions,
two 128-chunk pairs per instruction; operands in partition-major
layout so the 16 chunked target DMAs stream contiguously and overlap
the matmuls). DVE Q-max reduces psum into fp32 rows, then per-row
logsumexp and an approximate top-512 threshold (6-step bisection;
borderline mass folded in at exp(hi)) reduce each row to 6 stats; the
host finishes the per-row losses and averages. Rel err ~1e-4 (gate
is 2e-2).
"""
import sys

if "/opt/trn_rl_repo" not in sys.path:
    sys.path.insert(0, "/opt/trn_rl_repo")

import numpy as np

N_CORES = 8
B, Q, D = 1024, 32, 512
JQF = B * Q                    # 32768 target vectors (full, replicated)
NBLK = 512                     # jq per matmul / psum tile
JBLK = NBLK // Q               # 16 j columns per psum tile
NB = JQF // NBLK               # 64 jq blocks
NCHUNK = 16                    # target DMA chunks
N_ITERS = 6                    # bisection steps
NUM_HARD = B // 2              # 512
NEG_BIG = -1.0e30

_RUNNER = None


def _build():
    import concourse.bacc as bacc
    import concourse.mybir as mybir
    import concourse.tile as tile

    f32 = mybir.dt.float32
    f8 = mybir.dt.float8e4
    bf16 = mybir.dt.bfloat16
    i32 = mybir.dt.int32
    Alu = mybir.AluOpType
    Act = mybir.ActivationFunctionType
    X = mybir.AxisListType.X
    DR = mybir.MatmulPerfMode.DoubleRow

    nc = bacc.Bacc(None, target_bir_lowering=False, debug=False,
                   num_devices=N_CORES)

    fus_ap = nc.dram_tensor("fus8", [128, 2, 2, 128], f8, kind="ExternalInput").ap()
    tgt_ap = nc.dram_tensor("tgt8", [128, 2, 2, JQF], f8, kind="ExternalInput").ap()
    oneh_ap = nc.dram_tensor("onehot", [128, B], bf16, kind="ExternalInput").ap()
    out_ap = nc.dram_tensor("rowstats", [128, 6], f32, kind="ExternalOutput").ap()

    with tile.TileContext(nc) as tc:
        with (
            tc.tile_pool(name="fus", bufs=1) as fus_pool,
            tc.tile_pool(name="tgt", bufs=1) as tgt_pool,
            tc.tile_pool(name="res", bufs=1) as res_pool,
            tc.tile_pool(name="big", bufs=1) as big_pool,
            tc.tile_pool(name="small", bufs=1) as small_pool,
            tc.tile_pool(name="psum", bufs=8, space="PSUM") as psum_pool,
        ):
            # ---------- phase 1: my (128 x 1024) slab of v ----------
            fus = fus_pool.tile([128, 2, 2, 128], f8)
            nc.sync.dma_start(fus[:], fus_ap[:])
            oneh = big_pool.tile([128, B], bf16)
            nc.sync.dma_start(oneh[:], oneh_ap[:])
            # two half tiles: keeps the pair-dim stride (JQF/2) inside the
            # 16-bit ISA step field that a single [.., JQF] tile overflows
            HJ = JQF // 2
            tgt = [tgt_pool.tile([128, 2, 2, HJ], f8, name=f"tgt{h}",
                                 tag=f"tgt{h}") for h in range(2)]
            CH = HJ // (NCHUNK // 2)
            for h in range(2):
                for ck in range(NCHUNK // 2):
                    nc.sync.dma_start(
                        tgt[h][:, :, :, ck * CH:(ck + 1) * CH],
                        tgt_ap[:, :, :, h * HJ + ck * CH:h * HJ + (ck + 1) * CH])

            V32 = big_pool.tile([128, B], f32)
            BH = NB // 2
            for b in range(NB):
                ps = psum_pool.tile([128, NBLK], f32)
                for kp in range(2):
                    nc.tensor.matmul(
                        ps[:],
                        fus[:, kp],
                        tgt[b // BH][:, kp, :,
                                     (b % BH) * NBLK:(b % BH + 1) * NBLK],
                        start=(kp == 0),
                        stop=(kp == 1),
                        perf_mode=DR,
                    )
                nc.vector.reduce_max(
                    V32[:, b * JBLK:(b + 1) * JBLK],
                    ps.rearrange("p (j q) -> p j q", q=Q),
                    axis=X,
                )

            # ---------- phase 2: per-row stats ----------
            Vmask = big_pool.tile([128, B], f32)
            E = big_pool.tile([128, B], bf16)
            junk = big_pool.tile([128, B], bf16)
            junkf = big_pool.tile([128, B], f32)

            outs = res_pool.tile([128, 6], f32)  # m pos sumfull sumsel cnt_hi hi
            m = outs[:, 0:1]
            pos = outs[:, 1:2]
            sumfull = outs[:, 2:3]
            sumsel = outs[:, 3:4]
            cnt_hi = outs[:, 4:5]

            def sm(name, dt=f32):
                return small_pool.tile([128, 1], dt, name=name, tag=name)

            negm, lo, hi, mid, cnt = (sm(n) for n in "negm lo hi mid cnt".split())
            upd = sm("upd", i32)
            updn = sm("updn", i32)

            nc.vector.reduce_max(m, V32[:], axis=X)
            nc.vector.tensor_scalar_mul(negm[:], m, -1.0)
            nc.vector.tensor_reduce(lo[:], V32[:], axis=X, op=Alu.min)
            nc.vector.tensor_scalar_add(lo[:], lo[:], -1.0)
            nc.vector.tensor_copy(hi[:], m)

            # Vmask = V - 1e30 * onehot
            nc.vector.scalar_tensor_tensor(
                Vmask[:], oneh[:], NEG_BIG, V32[:], op0=Alu.mult, op1=Alu.add)

            # E = exp(V - m), sumfull = sum_j E  (scalar engine, concurrent)
            nc.scalar.activation(E[:], V32[:], Act.Exp, bias=negm[:], scale=1.0,
                                 accum_out=sumfull)

            # bisection for the top-512 threshold
            for _ in range(N_ITERS):
                nc.vector.tensor_add(mid[:], lo[:], hi[:])
                nc.vector.tensor_scalar_mul(mid[:], mid[:], 0.5)
                nc.vector.tensor_scalar(
                    junkf[:], Vmask[:], mid[:], None, op0=Alu.is_gt,
                    op1=Alu.add, accum_out=cnt[:])
                nc.vector.tensor_scalar(upd[:], cnt[:], float(NUM_HARD), None,
                                        op0=Alu.is_gt)
                nc.vector.tensor_scalar(updn[:], cnt[:], float(NUM_HARD), None,
                                        op0=Alu.is_le)
                nc.vector.copy_predicated(lo[:], upd[:], mid[:])
                nc.vector.copy_predicated(hi[:], updn[:], mid[:])

            # pos = sum(onehot * V)  (off the bisection chain)
            nc.vector.scalar_tensor_tensor(
                junk[:], oneh[:], 1.0, V32[:], op0=Alu.mult, op1=Alu.mult,
                accum_out=pos)
            # cnt_hi = #{v > hi};  sumsel = sum E over those entries
            nc.vector.tensor_scalar(
                junkf[:], Vmask[:], hi[:], None, op0=Alu.is_gt,
                op1=Alu.add, accum_out=cnt_hi)
            nc.vector.scalar_tensor_tensor(
                junkf[:], Vmask[:], hi[:], E[:], op0=Alu.is_gt, op1=Alu.mult,
                accum_out=sumsel)
            nc.vector.tensor_copy(outs[:, 5:6], hi[:])

            nc.sync.dma_start(out_ap[:], outs[:])

    nc.compile()
    return nc


def _get_nc():
    global _RUNNER
    if _RUNNER is None:
        _RUNNER = _build()
    return _RUNNER


def make_in_maps(fusion_feats, target_feats, temp):
    import ml_dtypes

    f8 = ml_dtypes.float8_e4m3
    fusion = np.asarray(fusion_feats, dtype=np.float32)
    target = np.asarray(target_feats, dtype=np.float32)
    scale = np.float32(1.0 / np.sqrt(float(np.asarray(temp))))
    # d -> (kp, pair, p): d = kp*256 + pair*128 + p; partition-major layout
    fusT = np.ascontiguousarray(
        (fusion * scale).T.reshape(2, 2, 128, B).transpose(2, 0, 1, 3)
    ).astype(f8)                                             # [128,2,2,B]
    tgt8 = np.ascontiguousarray(
        (target.reshape(JQF, D) * scale).T.reshape(2, 2, 128, JQF)
        .transpose(2, 0, 1, 3)
    ).astype(f8)                                             # [128,2,2,JQF]
    rows_per = B // N_CORES
    in_maps = []
    for c in range(N_CORES):
        fus8 = np.ascontiguousarray(
            fusT[:, :, :, c * rows_per:(c + 1) * rows_per])
        onehot = np.zeros((rows_per, B), dtype=ml_dtypes.bfloat16)
        onehot[np.arange(rows_per), c * rows_per + np.arange(rows_per)] = 1.0
        in_maps.append({"fus8": fus8, "tgt8": tgt8, "onehot": onehot})
    return in_maps


def combine(results):
    rows = np.concatenate([r["rowstats"] for r in results], axis=0)  # (1024, 6)
    m, pos, sumfull, sumsel, cnt_hi, hi = (rows[:, k].astype(np.float64)
                                           for k in range(6))
    epos = np.exp(pos - m)
    ehi = np.exp(hi - m)
    acc = epos + sumsel + (NUM_HARD - cnt_hi) * ehi
    loss_std = (m + np.log(sumfull) - pos).mean()
    loss_hard = (m + np.log(acc) - pos).mean()
    return np.asarray(loss_std + 0.5 * loss_hard, dtype=np.float32)


def kernel(fusion_feats, target_feats, temp):
    from concourse import bass_utils

    nc = _get_nc()
    in_maps = make_in_maps(fusion_feats, target_feats, temp)
    res = bass_utils.run_bass_kernel_spmd(nc, in_maps, list(range(N_CORES)))
    return combine(res.results)


# revision 11
# speedup vs baseline: 1.7063x; 1.1723x over previous
"""Hard-negative contrastive loss on 8 TRN2 NeuronCores (Bass/Tile).

Reference semantics (B=1024, Q=32, D=512, temp scalar):
    sim[i,j,q] = fusion[i] . target[j,q];  v[i,j] = max_q sim / temp
    loss = mean_i(lse_j(v[i,:]) - v[i,i])
         + 0.5 * mean_i(log(exp(pos) + sum exp(top512 offdiag)) - pos)

Sharding: OUTPUT rows i are split 128/core; the full fp8 target tensor
(16.8MB) is replicated to every core, so there is no inter-core
exchange at all (no collective, no skew coupling). Each core computes
its (128 x 1024) slab of v with fp8e4m3 DoubleRow matmuls
(sqrt(1/temp) folded into both operands host-side; d on partitions,
two 128-chunk pairs per instruction; operands in partition-major
layout so the 16 chunked target DMAs stream contiguously and overlap
the matmuls). DVE Q-max reduces two psum banks per op into fp32 rows.
Column-half stats (max/min/exp-sum/diag, biased per half) are emitted
as soon as each half of v is ready so they hide under remaining
matmuls; a 5-step bisection approximates the top-512 threshold
(borderline mass folded in at exp(hi), count-at-hi tracked in-loop).
Each row reduces to 10 stats; the host merges halves, finishes the
per-row losses, and averages. Rel err ~1e-4 (gate is 2e-2).
"""
import sys

if "/opt/trn_rl_repo" not in sys.path:
    sys.path.insert(0, "/opt/trn_rl_repo")

import numpy as np

N_CORES = 8
B, Q, D = 1024, 32, 512
JQF = B * Q                    # 32768 target vectors (full, replicated)
NBLK = 512                     # jq per matmul / psum bank
NB = JQF // NBLK               # 64 jq blocks
NB2 = NB // 2                  # 32 double-blocks (2 psum banks per reduce)
NCHUNK = 16                    # target DMA chunks
N_ITERS = 5                    # bisection steps
NUM_HARD = B // 2              # 512
NEG_BIG = -1.0e30

_RUNNER = None


def _build():
    import concourse.bacc as bacc
    import concourse.mybir as mybir
    import concourse.tile as tile

    f32 = mybir.dt.float32
    f8 = mybir.dt.float8e4
    bf16 = mybir.dt.bfloat16
    i32 = mybir.dt.int32
    Alu = mybir.AluOpType
    Act = mybir.ActivationFunctionType
    X = mybir.AxisListType.X
    DR = mybir.MatmulPerfMode.DoubleRow

    nc = bacc.Bacc(None, target_bir_lowering=False, debug=False,
                   num_devices=N_CORES)

    fus_ap = nc.dram_tensor("fus8", [128, 2, 2, 128], f8, kind="ExternalInput").ap()
    tgt_ap = nc.dram_tensor("tgt8", [128, 2, 2, JQF], f8, kind="ExternalInput").ap()
    oneh_ap = nc.dram_tensor("onehot", [128, B], bf16, kind="ExternalInput").ap()
    out_ap = nc.dram_tensor("rowstats", [128, 10], f32, kind="ExternalOutput").ap()

    with tile.TileContext(nc) as tc:
        with (
            tc.tile_pool(name="fus", bufs=1) as fus_pool,
            tc.tile_pool(name="tgt", bufs=1) as tgt_pool,
            tc.tile_pool(name="res", bufs=1) as res_pool,
            tc.tile_pool(name="big", bufs=1) as big_pool,
            tc.tile_pool(name="small", bufs=1) as small_pool,
            tc.tile_pool(name="psum", bufs=4, space="PSUM") as psum_pool,
        ):
            # ---------- phase 1: my (128 x 1024) slab of v ----------
            fus = fus_pool.tile([128, 2, 2, 128], f8)
            nc.sync.dma_start(fus[:], fus_ap[:])
            # two half tiles: keeps the pair-dim stride (JQF/2) inside the
            # 16-bit ISA step field that a single [.., JQF] tile overflows
            HJ = JQF // 2
            tgt = [tgt_pool.tile([128, 2, 2, HJ], f8, name=f"tgt{h}",
                                 tag=f"tgt{h}") for h in range(2)]
            CH = HJ // (NCHUNK // 2)
            for h in range(2):
                for ck in range(NCHUNK // 2):
                    nc.sync.dma_start(
                        tgt[h][:, :, :, ck * CH:(ck + 1) * CH],
                        tgt_ap[:, :, :, h * HJ + ck * CH:h * HJ + (ck + 1) * CH])
            oneh = big_pool.tile([128, B], bf16)
            nc.sync.dma_start(oneh[:], oneh_ap[:])

            V32 = big_pool.tile([128, B], f32)
            Vmask = big_pool.tile([128, B], f32)
            E = big_pool.tile([128, B], bf16)
            junk = big_pool.tile([128, B], bf16)
            junkf = big_pool.tile([128, B], f32)

            outs = res_pool.tile([128, 10], f32)
            # columns: mA mB posA posB sfA sfB ssA ssB cnt_hi hi
            mh = [outs[:, 0:1], outs[:, 1:2]]
            posh = [outs[:, 2:3], outs[:, 3:4]]
            sfh = [outs[:, 4:5], outs[:, 5:6]]
            ssh = [outs[:, 6:7], outs[:, 7:8]]

            def sm(name, dt=f32):
                return small_pool.tile([128, 1], dt, name=name, tag=name)

            negmh = [sm("negmA"), sm("negmB")]
            loh = [sm("loA"), sm("loB")]
            m, lo, hi, mid, cnt, cnt_hi = (
                sm(n) for n in "m lo hi mid cnt cnt_hi".split())
            upd = sm("upd", i32)
            updn = sm("updn", i32)

            def head_half(h):
                """Stats over column half h, hidden under remaining matmuls."""
                cs = slice(h * 512, (h + 1) * 512)
                nc.vector.reduce_max(mh[h], V32[:, cs], axis=X)
                nc.vector.tensor_reduce(loh[h][:], V32[:, cs], axis=X, op=Alu.min)
                nc.vector.tensor_scalar_mul(negmh[h][:], mh[h], -1.0)
                nc.vector.scalar_tensor_tensor(
                    Vmask[:, cs], oneh[:, cs], NEG_BIG, V32[:, cs],
                    op0=Alu.mult, op1=Alu.add)
                nc.vector.scalar_tensor_tensor(
                    junk[:, cs], oneh[:, cs], 1.0, V32[:, cs],
                    op0=Alu.mult, op1=Alu.mult, accum_out=posh[h])
                nc.scalar.activation(E[:, cs], V32[:, cs], Act.Exp,
                                     bias=negmh[h][:], scale=1.0,
                                     accum_out=sfh[h])

            for b2 in range(NB2):
                ps = psum_pool.tile([128, 2, NBLK], f32)
                for t in range(2):
                    b = b2 * 2 + t
                    half, bb = b // NB2, b % NB2
                    for kp in range(2):
                        nc.tensor.matmul(
                            ps[:, t],
                            fus[:, kp],
                            tgt[half][:, kp, :, bb * NBLK:(bb + 1) * NBLK],
                            start=(kp == 0),
                            stop=(kp == 1),
                            perf_mode=DR,
                        )
                nc.vector.reduce_max(
                    V32[:, b2 * 32:(b2 + 1) * 32],
                    ps.rearrange("p t (j q) -> p t j q", q=Q),
                    axis=X,
                )
                if b2 == NB2 // 2 - 1:
                    head_half(0)
            head_half(1)

            # ---------- phase 2: bisection on the full rows ----------
            nc.vector.tensor_tensor(m[:], mh[0], mh[1], op=Alu.max)
            nc.vector.tensor_tensor(lo[:], loh[0][:], loh[1][:], op=Alu.min)
            nc.vector.tensor_scalar_add(lo[:], lo[:], -1.0)
            nc.vector.tensor_copy(hi[:], m[:])
            nc.vector.memset(cnt_hi[:], 0.0)

            for _ in range(N_ITERS):
                nc.vector.tensor_add(mid[:], lo[:], hi[:])
                nc.vector.tensor_scalar_mul(mid[:], mid[:], 0.5)
                nc.vector.tensor_scalar(
                    junkf[:], Vmask[:], mid[:], None, op0=Alu.is_gt,
                    op1=Alu.add, accum_out=cnt[:])
                nc.vector.tensor_scalar(upd[:], cnt[:], float(NUM_HARD), None,
                                        op0=Alu.is_gt)
                nc.vector.tensor_scalar(updn[:], cnt[:], float(NUM_HARD), None,
                                        op0=Alu.is_le)
                nc.vector.copy_predicated(lo[:], upd[:], mid[:])
                nc.vector.copy_predicated(hi[:], updn[:], mid[:])
                nc.vector.copy_predicated(cnt_hi[:], updn[:], cnt[:])

            # sumsel per half = sum E over entries with v > hi
            for h in range(2):
                cs = slice(h * 512, (h + 1) * 512)
                nc.vector.scalar_tensor_tensor(
                    junkf[:, cs], Vmask[:, cs], hi[:], E[:, cs],
                    op0=Alu.is_gt, op1=Alu.mult, accum_out=ssh[h])
            nc.vector.tensor_copy(outs[:, 8:9], cnt_hi[:])
            nc.vector.tensor_copy(outs[:, 9:10], hi[:])

            nc.sync.dma_start(out_ap[:], outs[:])

    nc.compile()
    return nc


def _get_nc():
    global _RUNNER
    if _RUNNER is None:
        _RUNNER = _build()
    return _RUNNER


def make_in_maps(fusion_feats, target_feats, temp):
    import ml_dtypes

    f8 = ml_dtypes.float8_e4m3
    fusion = np.asarray(fusion_feats, dtype=np.float32)
    target = np.asarray(target_feats, dtype=np.float32)
    scale = np.float32(1.0 / np.sqrt(float(np.asarray(temp))))
    # d -> (kp, pair, p): d = kp*256 + pair*128 + p; partition-major layout
    fusT = np.ascontiguousarray(
        (fusion * scale).T.reshape(2, 2, 128, B).transpose(2, 0, 1, 3)
    ).astype(f8)                                             # [128,2,2,B]
    tgt8 = np.ascontiguousarray(
        (target.reshape(JQF, D) * scale).T.reshape(2, 2, 128, JQF)
        .transpose(2, 0, 1, 3)
    ).astype(f8)                                             # [128,2,2,JQF]
    rows_per = B // N_CORES
    in_maps = []
    for c in range(N_CORES):
        fus8 = np.ascontiguousarray(
            fusT[:, :, :, c * rows_per:(c + 1) * rows_per])
        onehot = np.zeros((rows_per, B), dtype=ml_dtypes.bfloat16)
        onehot[np.arange(rows_per), c * rows_per + np.arange(rows_per)] = 1.0
        in_maps.append({"fus8": fus8, "tgt8": tgt8, "onehot": onehot})
    return in_maps


def combine(results):
    rows = np.concatenate([r["rowstats"] for r in results], axis=0)  # (1024,10)
    (mA, mB, posA, posB, sfA, sfB, ssA, ssB, cnt_hi, hi) = (
        rows[:, k].astype(np.float64) for k in range(10))
    m = np.maximum(mA, mB)
    wA, wB = np.exp(mA - m), np.exp(mB - m)
    sumfull = sfA * wA + sfB * wB
    sumsel = ssA * wA + ssB * wB
    pos = posA + posB
    epos = np.exp(pos - m)
    ehi = np.exp(hi - m)
    acc = epos + sumsel + (NUM_HARD - cnt_hi) * ehi
    loss_std = (m + np.log(sumfull) - pos).mean()
    loss_hard = (m + np.log(acc) - pos).mean()
    return np.asarray(loss_std + 0.5 * loss_hard, dtype=np.float32)


def kernel(fusion_feats, target_feats, temp):
    from concourse import bass_utils

    nc = _get_nc()
    in_maps = make_in_maps(fusion_feats, target_feats, temp)
    res = bass_utils.run_bass_kernel_spmd(nc, in_maps, list(range(N_CORES)))
    return combine(res.results)


# revision 16
# speedup vs baseline: 1.7576x; 1.0301x over previous
"""Hard-negative contrastive loss on 8 TRN2 NeuronCores (Bass/Tile).

Reference semantics (B=1024, Q=32, D=512, temp scalar):
    sim[i,j,q] = fusion[i] . target[j,q];  v[i,j] = max_q sim / temp
    loss = mean_i(lse_j(v[i,:]) - v[i,i])
         + 0.5 * mean_i(log(exp(pos) + sum exp(top512 offdiag)) - pos)

Sharding: OUTPUT rows i are split 128/core; the full fp8 target tensor
(16.8MB) is replicated to every core, so there is no inter-core
exchange at all (no collective, no skew coupling). Each core computes
its (128 x 1024) slab of v with fp8e4m3 DoubleRow matmuls
(sqrt(1/temp) folded into both operands host-side; d on partitions,
two 128-chunk pairs per instruction; operands in partition-major
layout so the 16 chunked target DMAs stream contiguously and overlap
the matmuls). DVE Q-max reduces two psum banks per op into fp32 rows.
Column-half stats (max/min/exp-sum/diag, biased per half) are emitted
as soon as each half of v is ready so they hide under remaining
matmuls; a 5-step bisection approximates the top-512 threshold
(borderline mass folded in at exp(hi), count-at-hi tracked in-loop).
Each row reduces to 10 stats; the host merges halves, finishes the
per-row losses, and averages. Rel err ~1e-4 (gate is 2e-2).
"""
import sys

if "/opt/trn_rl_repo" not in sys.path:
    sys.path.insert(0, "/opt/trn_rl_repo")

import numpy as np

N_CORES = 8
B, Q, D = 1024, 32, 512
JQF = B * Q                    # 32768 target vectors (full, replicated)
NBLK = 512                     # jq per matmul / psum bank
NB = JQF // NBLK               # 64 jq blocks
NB2 = NB // 2                  # 32 double-blocks (2 psum banks per reduce)
NCHUNK = 16                    # target DMA chunks
N_ITERS = 5                    # bisection steps
NUM_HARD = B // 2              # 512
NEG_BIG = -1.0e30

_RUNNER = None


def _build():
    import concourse.bacc as bacc
    import concourse.mybir as mybir
    import concourse.tile as tile

    f32 = mybir.dt.float32
    f8 = mybir.dt.float8e4
    bf16 = mybir.dt.bfloat16
    i32 = mybir.dt.int32
    Alu = mybir.AluOpType
    Act = mybir.ActivationFunctionType
    X = mybir.AxisListType.X
    DR = mybir.MatmulPerfMode.DoubleRow

    nc = bacc.Bacc(None, target_bir_lowering=False, debug=False,
                   num_devices=N_CORES)

    fus_ap = nc.dram_tensor("fus8", [128, 2, 2, 128], f8, kind="ExternalInput").ap()
    tgt_ap = nc.dram_tensor("tgt8", [128, NCHUNK, 2, 2, JQF // NCHUNK], f8,
                            kind="ExternalInput").ap()
    oneh_ap = nc.dram_tensor("onehot", [128, B], bf16, kind="ExternalInput").ap()
    out_ap = nc.dram_tensor("rowstats", [128, 10], f32, kind="ExternalOutput").ap()

    with tile.TileContext(nc) as tc:
        with (
            tc.tile_pool(name="fus", bufs=1) as fus_pool,
            tc.tile_pool(name="tgt", bufs=1) as tgt_pool,
            tc.tile_pool(name="res", bufs=1) as res_pool,
            tc.tile_pool(name="big", bufs=1) as big_pool,
            tc.tile_pool(name="small", bufs=1) as small_pool,
            tc.tile_pool(name="psum", bufs=2, space="PSUM") as psum_pool,
        ):
            # ---------- phase 1: my (128 x 1024) slab of v ----------
            fus = fus_pool.tile([128, 2, 2, 128], f8)
            nc.sync.dma_start(fus[:], fus_ap[:])
            # chunk-major layout: each DMA chunk is one contiguous
            # 8KB-per-partition block (128 descriptors), and the pair-dim
            # stride (CH) stays inside the 16-bit ISA step field
            CH = JQF // NCHUNK
            tgt = tgt_pool.tile([128, NCHUNK, 2, 2, CH], f8)
            for ck in range(NCHUNK):
                nc.sync.dma_start(tgt[:, ck], tgt_ap[:, ck])
            oneh = big_pool.tile([128, B], bf16)

            V32 = big_pool.tile([128, B], f32)
            Vmask = big_pool.tile([128, B], f32)
            E = big_pool.tile([128, B], bf16)
            junk = big_pool.tile([128, B], bf16)
            junkf = big_pool.tile([128, B], f32)

            outs = res_pool.tile([128, 10], f32)
            # columns: mA mB posA posB sfA sfB ssA ssB cnt_hi hi
            mh = [outs[:, 0:1], outs[:, 1:2]]
            posh = [outs[:, 2:3], outs[:, 3:4]]
            sfh = [outs[:, 4:5], outs[:, 5:6]]
            ssh = [outs[:, 6:7], outs[:, 7:8]]

            def sm(name, dt=f32):
                return small_pool.tile([128, 1], dt, name=name, tag=name)

            negmh = [sm("negmA"), sm("negmB")]
            loh = [sm("loA"), sm("loB")]
            m, lo, hi, mid, cnt, cnt_hi = (
                sm(n) for n in "m lo hi mid cnt cnt_hi".split())
            upd = sm("upd", i32)
            updn = sm("updn", i32)

            def head_half(h):
                """Stats over column half h, hidden under remaining matmuls."""
                cs = slice(h * 512, (h + 1) * 512)
                nc.vector.reduce_max(mh[h], V32[:, cs], axis=X)
                nc.vector.tensor_reduce(loh[h][:], V32[:, cs], axis=X, op=Alu.min)
                nc.vector.tensor_scalar_mul(negmh[h][:], mh[h], -1.0)
                nc.vector.scalar_tensor_tensor(
                    Vmask[:, cs], oneh[:, cs], NEG_BIG, V32[:, cs],
                    op0=Alu.mult, op1=Alu.add)
                nc.vector.scalar_tensor_tensor(
                    junk[:, cs], oneh[:, cs], 1.0, V32[:, cs],
                    op0=Alu.mult, op1=Alu.mult, accum_out=posh[h])
                nc.scalar.activation(E[:, cs], V32[:, cs], Act.Exp,
                                     bias=negmh[h][:], scale=1.0,
                                     accum_out=sfh[h])

            BPC = CH // NBLK                 # 4 jq blocks per chunk
            NB4 = NB // 4                    # 16 quad-blocks (4 banks/reduce)
            for b4 in range(NB4):
                ps = psum_pool.tile([128, 4, NBLK], f32)
                for t in range(4):
                    b = b4 * 4 + t
                    ck, bb = b // BPC, b % BPC
                    for kp in range(2):
                        nc.tensor.matmul(
                            ps[:, t],
                            fus[:, kp],
                            tgt[:, ck, kp, :, bb * NBLK:(bb + 1) * NBLK],
                            start=(kp == 0),
                            stop=(kp == 1),
                            perf_mode=DR,
                        )
                nc.vector.reduce_max(
                    V32[:, b4 * 64:(b4 + 1) * 64],
                    ps.rearrange("p t (j q) -> p t j q", q=Q),
                    axis=X,
                )
                if b4 == 1:
                    nc.sync.dma_start(oneh[:], oneh_ap[:])
                if b4 == NB4 // 2 - 1:
                    head_half(0)
            head_half(1)

            # ---------- phase 2: bisection on the full rows ----------
            nc.vector.tensor_tensor(m[:], mh[0], mh[1], op=Alu.max)
            nc.vector.tensor_tensor(lo[:], loh[0][:], loh[1][:], op=Alu.min)
            nc.vector.tensor_scalar_add(lo[:], lo[:], -1.0)
            nc.vector.tensor_copy(hi[:], m[:])
            nc.vector.memset(cnt_hi[:], 0.0)

            for _ in range(N_ITERS):
                nc.vector.tensor_add(mid[:], lo[:], hi[:])
                nc.vector.tensor_scalar_mul(mid[:], mid[:], 0.5)
                nc.vector.tensor_scalar(
                    junkf[:], Vmask[:], mid[:], None, op0=Alu.is_gt,
                    op1=Alu.add, accum_out=cnt[:])
                nc.vector.tensor_scalar(upd[:], cnt[:], float(NUM_HARD), None,
                                        op0=Alu.is_gt)
                nc.vector.tensor_scalar(updn[:], cnt[:], float(NUM_HARD), None,
                                        op0=Alu.is_le)
                nc.vector.copy_predicated(lo[:], upd[:], mid[:])
                nc.vector.copy_predicated(hi[:], updn[:], mid[:])
                nc.vector.copy_predicated(cnt_hi[:], updn[:], cnt[:])

            # sumsel per half = sum E over entries with v > hi
            for h in range(2):
                cs = slice(h * 512, (h + 1) * 512)
                nc.vector.scalar_tensor_tensor(
                    junkf[:, cs], Vmask[:, cs], hi[:], E[:, cs],
                    op0=Alu.is_gt, op1=Alu.mult, accum_out=ssh[h])
            nc.vector.tensor_copy(outs[:, 8:9], cnt_hi[:])
            nc.vector.tensor_copy(outs[:, 9:10], hi[:])

            nc.sync.dma_start(out_ap[:], outs[:])

    nc.compile()
    return nc


def _get_nc():
    global _RUNNER
    if _RUNNER is None:
        _RUNNER = _build()
    return _RUNNER


def make_in_maps(fusion_feats, target_feats, temp):
    import ml_dtypes

    f8 = ml_dtypes.float8_e4m3
    fusion = np.asarray(fusion_feats, dtype=np.float32)
    target = np.asarray(target_feats, dtype=np.float32)
    scale = np.float32(1.0 / np.sqrt(float(np.asarray(temp))))
    # d -> (kp, pair, p): d = kp*256 + pair*128 + p; partition-major layout
    fusT = np.ascontiguousarray(
        (fusion * scale).T.reshape(2, 2, 128, B).transpose(2, 0, 1, 3)
    ).astype(f8)                                             # [128,2,2,B]
    CH = JQF // NCHUNK
    tgt8 = np.ascontiguousarray(
        (target.reshape(JQF, D) * scale).T.reshape(2, 2, 128, NCHUNK, CH)
        .transpose(2, 3, 0, 1, 4)
    ).astype(f8)                                             # [128,NCHUNK,2,2,CH]
    rows_per = B // N_CORES
    in_maps = []
    for c in range(N_CORES):
        fus8 = np.ascontiguousarray(
            fusT[:, :, :, c * rows_per:(c + 1) * rows_per])
        onehot = np.zeros((rows_per, B), dtype=ml_dtypes.bfloat16)
        onehot[np.arange(rows_per), c * rows_per + np.arange(rows_per)] = 1.0
        in_maps.append({"fus8": fus8, "tgt8": tgt8, "onehot": onehot})
    return in_maps


def combine(results):
    rows = np.concatenate([r["rowstats"] for r in results], axis=0)  # (1024,10)
    (mA, mB, posA, posB, sfA, sfB, ssA, ssB, cnt_hi, hi) = (
        rows[:, k].astype(np.float64) for k in range(10))
    m = np.maximum(mA, mB)
    wA, wB = np.exp(mA - m), np.exp(mB - m)
    sumfull = sfA * wA + sfB * wB
    sumsel = ssA * wA + ssB * wB
    pos = posA + posB
    epos = np.exp(pos - m)
    ehi = np.exp(hi - m)
    acc = epos + sumsel + (NUM_HARD - cnt_hi) * ehi
    loss_std = (m + np.log(sumfull) - pos).mean()
    loss_hard = (m + np.log(acc) - pos).mean()
    return np.asarray(loss_std + 0.5 * loss_hard, dtype=np.float32)


def kernel(fusion_feats, target_feats, temp):
    from concourse import bass_utils

    nc = _get_nc()
    in_maps = make_in_maps(fusion_feats, target_feats, temp)
    res = bass_utils.run_bass_kernel_spmd(nc, in_maps, list(range(N_CORES)))
    return combine(res.results)


# revision 17
# speedup vs baseline: 1.9495x; 1.1092x over previous
"""Hard-negative contrastive loss on 8 TRN2 NeuronCores (Bass/Tile).

Reference semantics (B=1024, Q=32, D=512, temp scalar):
    sim[i,j,q] = fusion[i] . target[j,q];  v[i,j] = max_q sim / temp
    loss = mean_i(lse_j(v[i,:]) - v[i,i])
         + 0.5 * mean_i(log(exp(pos) + sum exp(top512 offdiag)) - pos)

Sharding: OUTPUT rows i are split 128/core; the full fp8 target tensor
(16.8MB) is replicated to every core, so there is no inter-core
exchange at all (no collective, no skew coupling). Each core computes
its (128 x 1024) slab of v with fp8e4m3 DoubleRow matmuls
(sqrt(1/temp) folded into both operands host-side; d on partitions,
two 128-chunk pairs per instruction; operands in partition-major
layout so the 16 chunked target DMAs stream contiguously and overlap
the matmuls). DVE Q-max reduces two psum banks per op into fp32 rows.
Column-half stats (max/min/exp-sum/diag, biased per half) are emitted
as soon as each half of v is ready so they hide under remaining
matmuls; a 5-step bisection approximates the top-512 threshold
(borderline mass folded in at exp(hi), count-at-hi tracked in-loop).
Each row reduces to 10 stats; the host merges halves, finishes the
per-row losses, and averages. Rel err ~1e-4 (gate is 2e-2).
"""
import sys

if "/opt/trn_rl_repo" not in sys.path:
    sys.path.insert(0, "/opt/trn_rl_repo")

import numpy as np

N_CORES = 8
B, Q, D = 1024, 32, 512
JQF = B * Q                    # 32768 target vectors (full, replicated)
NBLK = 512                     # jq per matmul / psum bank
NB = JQF // NBLK               # 64 jq blocks
NB2 = NB // 2                  # 32 double-blocks (2 psum banks per reduce)
NCHUNK = 16                    # target DMA chunks
N_ITERS = 4                    # bisection steps
NUM_HARD = B // 2              # 512
NEG_BIG = -1.0e30

_RUNNER = None


def _build():
    import concourse.bacc as bacc
    import concourse.mybir as mybir
    import concourse.tile as tile

    f32 = mybir.dt.float32
    f8 = mybir.dt.float8e4
    bf16 = mybir.dt.bfloat16
    i32 = mybir.dt.int32
    Alu = mybir.AluOpType
    Act = mybir.ActivationFunctionType
    X = mybir.AxisListType.X
    DR = mybir.MatmulPerfMode.DoubleRow

    nc = bacc.Bacc(None, target_bir_lowering=False, debug=False,
                   num_devices=N_CORES)

    fus_ap = nc.dram_tensor("fus8", [128, 2, 2, 128], f8, kind="ExternalInput").ap()
    tgt_ap = nc.dram_tensor("tgt8", [128, NCHUNK, 2, 2, JQF // NCHUNK], f8,
                            kind="ExternalInput").ap()
    oneh_ap = nc.dram_tensor("onehot", [128, B], bf16, kind="ExternalInput").ap()
    out_ap = nc.dram_tensor("rowstats", [128, 10], f32, kind="ExternalOutput").ap()

    with tile.TileContext(nc) as tc:
        with (
            tc.tile_pool(name="fus", bufs=1) as fus_pool,
            tc.tile_pool(name="tgt", bufs=1) as tgt_pool,
            tc.tile_pool(name="res", bufs=1) as res_pool,
            tc.tile_pool(name="big", bufs=1) as big_pool,
            tc.tile_pool(name="small", bufs=1) as small_pool,
            tc.tile_pool(name="psum", bufs=2, space="PSUM") as psum_pool,
        ):
            # ---------- phase 1: my (128 x 1024) slab of v ----------
            fus = fus_pool.tile([128, 2, 2, 128], f8)
            nc.sync.dma_start(fus[:], fus_ap[:])
            # chunk-major layout: each DMA chunk is one contiguous
            # 8KB-per-partition block (128 descriptors), and the pair-dim
            # stride (CH) stays inside the 16-bit ISA step field
            CH = JQF // NCHUNK
            tgt = tgt_pool.tile([128, NCHUNK, 2, 2, CH], f8)
            for ck in range(NCHUNK):
                nc.sync.dma_start(tgt[:, ck], tgt_ap[:, ck])
            oneh = big_pool.tile([128, B], bf16)

            V32 = big_pool.tile([128, B], f32)
            Vmask = big_pool.tile([128, B], f32)
            E = big_pool.tile([128, B], bf16)
            junk = big_pool.tile([128, B], bf16)
            junkf = big_pool.tile([128, B], f32)

            outs = res_pool.tile([128, 10], f32)
            # columns: mA mB posA posB sfA sfB ssA ssB cnt_hi hi
            mh = [outs[:, 0:1], outs[:, 1:2]]
            posh = [outs[:, 2:3], outs[:, 3:4]]
            sfh = [outs[:, 4:5], outs[:, 5:6]]
            ssh = [outs[:, 6:7], outs[:, 7:8]]

            def sm(name, dt=f32):
                return small_pool.tile([128, 1], dt, name=name, tag=name)

            negmh = [sm("negmA"), sm("negmB")]
            loh = [sm("loA"), sm("loB")]
            m, lo, hi, mid, cnt, cnt_hi = (
                sm(n) for n in "m lo hi mid cnt cnt_hi".split())
            upd = sm("upd", i32)
            updn = sm("updn", i32)

            def head_half(h):
                """Stats over column span h, hidden under remaining matmuls
                (span A = 3/4 of columns, span B = the last 1/4)."""
                cs = slice(0, 768) if h == 0 else slice(768, 1024)
                nc.vector.reduce_max(mh[h], V32[:, cs], axis=X)
                nc.vector.tensor_reduce(loh[h][:], V32[:, cs], axis=X, op=Alu.min)
                nc.vector.tensor_scalar_mul(negmh[h][:], mh[h], -1.0)
                nc.vector.scalar_tensor_tensor(
                    Vmask[:, cs], oneh[:, cs], NEG_BIG, V32[:, cs],
                    op0=Alu.mult, op1=Alu.add)
                nc.vector.scalar_tensor_tensor(
                    junk[:, cs], oneh[:, cs], 1.0, V32[:, cs],
                    op0=Alu.mult, op1=Alu.mult, accum_out=posh[h])
                nc.scalar.activation(E[:, cs], V32[:, cs], Act.Exp,
                                     bias=negmh[h][:], scale=1.0,
                                     accum_out=sfh[h])

            BPC = CH // NBLK                 # 4 jq blocks per chunk
            NB4 = NB // 4                    # 16 quad-blocks (4 banks/reduce)
            for b4 in range(NB4):
                ps = psum_pool.tile([128, 4, NBLK], f32)
                for t in range(4):
                    b = b4 * 4 + t
                    ck, bb = b // BPC, b % BPC
                    for kp in range(2):
                        nc.tensor.matmul(
                            ps[:, t],
                            fus[:, kp],
                            tgt[:, ck, kp, :, bb * NBLK:(bb + 1) * NBLK],
                            start=(kp == 0),
                            stop=(kp == 1),
                            perf_mode=DR,
                        )
                nc.vector.reduce_max(
                    V32[:, b4 * 64:(b4 + 1) * 64],
                    ps.rearrange("p t (j q) -> p t j q", q=Q),
                    axis=X,
                )
                if b4 == 1:
                    nc.sync.dma_start(oneh[:], oneh_ap[:])
                if b4 == 3 * NB4 // 4 - 1:
                    head_half(0)
            head_half(1)

            # ---------- phase 2: bisection on the full rows ----------
            nc.vector.tensor_tensor(m[:], mh[0], mh[1], op=Alu.max)
            nc.vector.tensor_tensor(lo[:], loh[0][:], loh[1][:], op=Alu.min)
            nc.vector.tensor_scalar_add(lo[:], lo[:], -1.0)
            nc.vector.tensor_copy(hi[:], m[:])
            nc.vector.memset(cnt_hi[:], 0.0)

            for _ in range(N_ITERS):
                nc.vector.tensor_add(mid[:], lo[:], hi[:])
                nc.vector.tensor_scalar_mul(mid[:], mid[:], 0.5)
                nc.vector.tensor_scalar(
                    junkf[:], Vmask[:], mid[:], None, op0=Alu.is_gt,
                    op1=Alu.add, accum_out=cnt[:])
                nc.vector.tensor_scalar(upd[:], cnt[:], float(NUM_HARD), None,
                                        op0=Alu.is_gt)
                nc.vector.tensor_scalar(updn[:], cnt[:], float(NUM_HARD), None,
                                        op0=Alu.is_le)
                nc.vector.copy_predicated(lo[:], upd[:], mid[:])
                nc.vector.copy_predicated(hi[:], updn[:], mid[:])
                nc.vector.copy_predicated(cnt_hi[:], updn[:], cnt[:])

            # sumsel per half = sum E over entries with v > hi
            for h in range(2):
                cs = slice(0, 768) if h == 0 else slice(768, 1024)
                nc.vector.scalar_tensor_tensor(
                    junkf[:, cs], Vmask[:, cs], hi[:], E[:, cs],
                    op0=Alu.is_gt, op1=Alu.mult, accum_out=ssh[h])
            nc.vector.tensor_copy(outs[:, 8:9], cnt_hi[:])
            nc.vector.tensor_copy(outs[:, 9:10], hi[:])

            nc.sync.dma_start(out_ap[:], outs[:])

    nc.compile()
    return nc


def _get_nc():
    global _RUNNER
    if _RUNNER is None:
        _RUNNER = _build()
    return _RUNNER


def make_in_maps(fusion_feats, target_feats, temp):
    import ml_dtypes

    f8 = ml_dtypes.float8_e4m3
    fusion = np.asarray(fusion_feats, dtype=np.float32)
    target = np.asarray(target_feats, dtype=np.float32)
    scale = np.float32(1.0 / np.sqrt(float(np.asarray(temp))))
    # d -> (kp, pair, p): d = kp*256 + pair*128 + p; partition-major layout
    fusT = np.ascontiguousarray(
        (fusion * scale).T.reshape(2, 2, 128, B).transpose(2, 0, 1, 3)
    ).astype(f8)                                             # [128,2,2,B]
    CH = JQF // NCHUNK
    tgt8 = np.ascontiguousarray(
        (target.reshape(JQF, D) * scale).T.reshape(2, 2, 128, NCHUNK, CH)
        .transpose(2, 3, 0, 1, 4)
    ).astype(f8)                                             # [128,NCHUNK,2,2,CH]
    rows_per = B // N_CORES
    in_maps = []
    for c in range(N_CORES):
        fus8 = np.ascontiguousarray(
            fusT[:, :, :, c * rows_per:(c + 1) * rows_per])
        onehot = np.zeros((rows_per, B), dtype=ml_dtypes.bfloat16)
        onehot[np.arange(rows_per), c * rows_per + np.arange(rows_per)] = 1.0
        in_maps.append({"fus8": fus8, "tgt8": tgt8, "onehot": onehot})
    return in_maps


def combine(results):
    rows = np.concatenate([r["rowstats"] for r in results], axis=0)  # (1024,10)
    (mA, mB, posA, posB, sfA, sfB, ssA, ssB, cnt_hi, hi) = (
        rows[:, k].astype(np.float64) for k in range(10))
    m = np.maximum(mA, mB)
    wA, wB = np.exp(mA - m), np.exp(mB - m)
    sumfull = sfA * wA + sfB * wB
    sumsel = ssA * wA + ssB * wB
    pos = posA + posB
    epos = np.exp(pos - m)
    ehi = np.exp(hi - m)
    acc = epos + sumsel + (NUM_HARD - cnt_hi) * ehi
    loss_std = (m + np.log(sumfull) - pos).mean()
    loss_hard = (m + np.log(acc) - pos).mean()
    return np.asarray(loss_std + 0.5 * loss_hard, dtype=np.float32)


def kernel(fusion_feats, target_feats, temp):
    from concourse import bass_utils

    nc = _get_nc()
    in_maps = make_in_maps(fusion_feats, target_feats, temp)
    res = bass_utils.run_bass_kernel_spmd(nc, in_maps, list(range(N_CORES)))
    return combine(res.results)
